# revision 1
# baseline (speedup 1.0000x reference)
import numpy as np
import jax
import jax.numpy as jnp
from functools import partial

# KalmanNet gain network, data-parallel over batch on 8 NeuronCores.
# B=32768 sharded 8 ways (4096/core); per-d parameters replicated.
B, D, M, N = 32768, 16, 2, 1
NCORES = 8
EPS = 1e-6

_PARAM_KEYS = [
    'fc1_w', 'fc1_b', 'fc2_w', 'fc2_b', 'fc3_w', 'fc3_b', 'fc4_w', 'fc4_b',
    'fc5a_w', 'fc5a_b', 'fc5b_w', 'fc5b_b', 'fc6_w', 'fc6_b', 'fc7_w', 'fc7_b',
    'gru1_wih', 'gru1_whh', 'gru1_bih', 'gru1_bhh',
    'gru2_wih', 'gru2_whh', 'gru2_bih', 'gru2_bhh',
    'gru3_wih', 'gru3_whh', 'gru3_bih', 'gru3_bhh',
]
_BATCH_KEYS = ['del_y_til', 'del_y', 'del_x_til', 'del_x_hat', 'Q', 'Sigma', 'S']


def _lin(x, w, b):
    return jnp.einsum('bdi,doi->bdo', x, w) + b


def _fc(x, w, b):
    return jax.nn.relu(_lin(x, w, b))


def _l2norm(x):
    nrm = jnp.sqrt(jnp.sum(x * x, axis=-1, keepdims=True))
    return x / jnp.maximum(nrm, EPS)


def _gru_step(x, h, wih, whh, bih, bhh):
    gi = jnp.einsum('bdi,dgi->bdg', x, wih) + bih
    gh = jnp.einsum('bdh,dgh->bdg', h, whh) + bhh
    ir, iz, i_n = jnp.split(gi, 3, axis=-1)
    hr, hz, h_n = jnp.split(gh, 3, axis=-1)
    r = jax.nn.sigmoid(ir + hr)
    z = jax.nn.sigmoid(iz + hz)
    cand = jnp.tanh(i_n + r * h_n)
    return (1.0 - z) * cand + z * h


def _forward(batch, params):
    (del_y_til, del_y, del_x_til, del_x_hat, Q, Sigma, S) = batch
    p = dict(zip(_PARAM_KEYS, params))
    in1 = _l2norm(_fc(del_x_hat, p['fc1_w'], p['fc1_b']))
    Qn = _gru_step(in1, Q, p['gru1_wih'], p['gru1_whh'],
                   p['gru1_bih'], p['gru1_bhh'])
    in2 = _l2norm(jnp.concatenate(
        [Qn, _fc(del_x_til, p['fc2_w'], p['fc2_b'])], axis=-1))
    Sigman = _gru_step(in2, Sigma, p['gru2_wih'], p['gru2_whh'],
                       p['gru2_bih'], p['gru2_bhh'])
    in3 = _l2norm(jnp.concatenate([
        _fc(Sigman, p['fc3_w'], p['fc3_b']),
        _fc(jnp.concatenate([del_y_til, del_y], axis=-1),
            p['fc4_w'], p['fc4_b'])], axis=-1))
    Sn = _gru_step(in3, S, p['gru3_wih'], p['gru3_whh'],
                   p['gru3_bih'], p['gru3_bhh'])
    cat_ss = jnp.concatenate([Sigman, Sn], axis=-1)
    K = _lin(jax.nn.relu(_lin(cat_ss, p['fc5a_w'], p['fc5a_b'])),
             p['fc5b_w'], p['fc5b_b'])
    Sigma_next = _fc(jnp.concatenate(
        [Sigman, _fc(jnp.concatenate([Sn, K], axis=-1), p['fc6_w'], p['fc6_b'])],
        axis=-1), p['fc7_w'], p['fc7_b'])
    return jnp.concatenate([K, Qn, Sigma_next, Sn], axis=-1)


_pmapped = None
_param_cache = {}


def _get_pmapped():
    global _pmapped
    if _pmapped is None:
        devs = jax.devices()[:NCORES]
        _pmapped = jax.pmap(_forward, axis_name='cores', devices=devs)
    return _pmapped


def _device_params(inputs):
    # Replicate the small per-d parameter stack onto every core once;
    # reuse device buffers across calls when the host arrays are unchanged.
    key = tuple(id(inputs[k]) for k in _PARAM_KEYS)
    if _param_cache.get('key') != key:
        devs = jax.devices()[:NCORES]
        _param_cache['val'] = [
            jax.device_put_replicated(np.asarray(inputs[k]), devs)
            for k in _PARAM_KEYS]
        _param_cache['key'] = key
    return _param_cache['val']


def kernel(**inputs):
    # Pure data parallel: shard the batch axis across the 8 cores.
    batch = [np.asarray(inputs[k]).reshape(NCORES, B // NCORES,
                                           *inputs[k].shape[1:])
             for k in _BATCH_KEYS]
    out = _get_pmapped()(batch, _device_params(inputs))
    return np.asarray(out).reshape(B, D, 11)



# revision 45
# speedup vs baseline: 1.0016x; 1.0016x over previous
import numpy as np

# KalmanNet gain network on 8 trn2 NeuronCores via a Bass/Tile kernel.
# Data-parallel over batch: B=32768 -> 4096/core, processed as 512-column
# chunks in [feature-row, batch-column] layout (features packed across the
# 16 per-d networks onto SBUF partitions).
#
# Fast path assumes the recurrent states Q/Sigma/S are zero (they are, per
# the harness input spec); if any is nonzero we fall back to a jax pmap
# implementation that handles the general case.
#
# Row conventions (d = source index 0..15):
#   u tensors (40 wide):    global row g = 40*d + f, five 128-row tiles
#   gru1/2 gate blocks:     row = 16*h + d   (h = hidden unit 0..3)
#   gru3 gate blocks:       row = 16*g + d   (g = gate)
#   K output:               row = 2*d + k

B, D = 32768, 16
NCORES = 8
CH = 512            # columns per chunk (one PSUM bank of fp32)
EPS = 1e-6

F16 = np.float16
F32 = np.float32

_PARAM_KEYS = [
    'fc1_w', 'fc1_b', 'fc2_w', 'fc2_b', 'fc3_w', 'fc3_b', 'fc4_w', 'fc4_b',
    'fc5a_w', 'fc5a_b', 'fc5b_w', 'fc5b_b', 'fc6_w', 'fc6_b', 'fc7_w', 'fc7_b',
    'gru1_wih', 'gru1_whh', 'gru1_bih', 'gru1_bhh',
    'gru2_wih', 'gru2_whh', 'gru2_bih', 'gru2_bhh',
    'gru3_wih', 'gru3_whh', 'gru3_bih', 'gru3_bhh',
]

_NU_T = 5
_NV_T = (100 * D + 127) // 128  # 13


def _u_tile_span(t):
    dmin = (128 * t) // 40
    dmax = (128 * t + 127) // 40
    return dmin, dmax, 6 * dmin, 6 * dmax + 6


class _Pack:
    def __init__(self, dtype):
        self.dtype = dtype
        self.blocks = []
        self.col = 0
        self.slots = {}

    def add(self, name, mat):
        k, m = mat.shape
        assert k <= 128
        a = np.zeros((128, m), self.dtype)
        a[:k] = mat
        self.blocks.append(a)
        self.slots[name] = (k, self.col, m)
        self.col += m

    def pack(self):
        return np.ascontiguousarray(np.concatenate(self.blocks, axis=1))


def _build_packs(p):
    wp = _Pack(F16)
    bp = _Pack(F32)

    for t in range(_NU_T):
        dmin, dmax, klo, khi = _u_tile_span(t)
        K = khi - klo
        u1 = np.zeros((K, 128), F32)
        u2 = np.zeros((K, 128), F32)
        u4 = np.zeros((K, 128), F32)
        ss = np.zeros((128, 16), F32)
        g1A = np.zeros((128, 128), F32)
        g1B = np.zeros((128, 64), F32)
        g2A = np.zeros((128, 128), F32)
        g2B = np.zeros((128, 64), F32)
        g3 = np.zeros((128, 80), F32)
        bu1 = np.zeros(128, F32)
        bu2 = np.zeros(128, F32)
        bu4 = np.zeros(128, F32)
        for c in range(128):
            g = 128 * t + c
            d, fo = g // 40, g % 40
            base = 6 * (d - dmin)
            for i in range(2):
                u1[base + 4 + i, c] = p['fc1_w'][d, fo, i]
                u2[base + 2 + i, c] = p['fc2_w'][d, fo, i]
                u4[base + 0 + i, c] = p['fc4_w'][d, fo, i]
            ss[c, d] = 1.0
            for h in range(4):
                g1A[c, 16 * h + d] = p['gru1_wih'][d, h, fo]
                g1A[c, 64 + 16 * h + d] = p['gru1_wih'][d, 4 + h, fo]
                g1B[c, 16 * h + d] = p['gru1_wih'][d, 8 + h, fo]
                g2A[c, 16 * h + d] = p['gru2_wih'][d, h, 4 + fo]
                g2A[c, 64 + 16 * h + d] = p['gru2_wih'][d, 4 + h, 4 + fo]
                g2B[c, 16 * h + d] = p['gru2_wih'][d, 8 + h, 4 + fo]
            for gg in range(3):
                g3[c, 32 * gg + d] = p['gru3_wih'][d, gg, 1 + fo]
            bu1[c] = p['fc1_b'][d, fo]
            bu2[c] = p['fc2_b'][d, fo]
            bu4[c] = p['fc4_b'][d, fo]
        wp.add(f'u1_{t}', u1)
        wp.add(f'u2_{t}', u2)
        wp.add(f'u4_{t}', u4)
        wp.add(f'ss_{t}', ss)
        wp.add(f'g1A_{t}', g1A)
        wp.add(f'g1B_{t}', g1B)
        wp.add(f'g2A_{t}', g2A)
        wp.add(f'g2B_{t}', g2B)
        wp.add(f'g3_{t}', g3)
        bp.add(f'bu1_{t}', bu1[:, None])
        bp.add(f'bu2_{t}', bu2[:, None])
        bp.add(f'bu4_{t}', bu4[:, None])

    # Qn tile (rows 16*h + d)
    g2Aq = np.zeros((64, 128), F32)
    g2Bq = np.zeros((64, 64), F32)
    ssq = np.zeros((64, 16), F32)
    for d in range(D):
        for hin in range(4):
            k = 16 * hin + d
            ssq[k, d] = 1.0
            for h in range(4):
                g2Aq[k, 16 * h + d] = p['gru2_wih'][d, h, hin]
                g2Aq[k, 64 + 16 * h + d] = p['gru2_wih'][d, 4 + h, hin]
                g2Bq[k, 16 * h + d] = p['gru2_wih'][d, 8 + h, hin]
    wp.add('g2A_q', g2Aq)
    wp.add('g2B_q', g2Bq)
    wp.add('ss_q', ssq)

    # u3 = fc3 @ Sigman  (Sigman rows 16*i + d)
    fc3 = np.zeros((64, 16), F32)
    for d in range(D):
        for i in range(4):
            fc3[16 * i + d, d] = p['fc3_w'][d, 0, i]
    wp.add('fc3', fc3)
    g3u = np.zeros((16, 80), F32)
    ss3u = np.zeros((16, 16), F32)
    for d in range(D):
        ss3u[d, d] = 1.0
        for gg in range(3):
            g3u[d, 32 * gg + d] = p['gru3_wih'][d, gg, 0]
    wp.add('g3_u3', g3u)
    wp.add('ss_u3', ss3u)

    # fc5a / fc5b (cat = [Sigman(64: 16i+d), Sn(16: d)])
    for t in range(_NV_T):
        va = np.zeros((80, 128), F32)
        k5 = np.zeros((128, 32), F32)
        bv = np.zeros(128, F32)
        for c in range(128):
            g = 128 * t + c
            if g >= 100 * D:
                continue
            dv, j = g // 100, g % 100
            for i in range(4):
                va[16 * i + dv, c] = p['fc5a_w'][dv, j, i]
            va[64 + dv, c] = p['fc5a_w'][dv, j, 4]
            for kk in range(2):
                k5[c, 2 * dv + kk] = p['fc5b_w'][dv, kk, j]
            bv[c] = p['fc5a_b'][dv, j]
        wp.add(f'v_{t}', va)
        wp.add(f'k5_{t}', k5)
        bp.add(f'bv_{t}', bv[:, None])

    # fc6: input snk = [Sn(0:16: d), pad(16:32), K(32:64: 2d+k)];
    # out rows 16*o + d
    u6w = np.zeros((64, 64), F32)
    for d in range(D):
        for o in range(4):
            u6w[d, 16 * o + d] = p['fc6_w'][d, o, 0]
            for kk in range(2):
                u6w[32 + 2 * d + kk, 16 * o + d] = p['fc6_w'][d, o, 1 + kk]
    wp.add('u6w', u6w)
    # fc7: [Sigman(16i+d), u6(16h+d)]; out rows 16*o + d
    u7a = np.zeros((64, 64), F32)
    u7b = np.zeros((64, 64), F32)
    for d in range(D):
        for o in range(4):
            for i in range(4):
                u7a[16 * i + d, 16 * o + d] = p['fc7_w'][d, o, i]
                u7b[16 * i + d, 16 * o + d] = p['fc7_w'][d, o, 4 + i]
    wp.add('u7a', u7a)
    wp.add('u7b', u7b)

    def gate_bias(bih, bhh, h, zoff):
        rz = np.zeros(zoff + 16 * h, F32)
        bn = np.zeros(16 * h, F32)
        bhn = np.zeros(16 * h, F32)
        for d in range(D):
            for j in range(h):
                rz[16 * j + d] = bih[d, j] + bhh[d, j]
                rz[zoff + 16 * j + d] = bih[d, h + j] + bhh[d, h + j]
                bn[16 * j + d] = bih[d, 2 * h + j]
                bhn[16 * j + d] = bhh[d, 2 * h + j]
        return rz, bn, bhn

    for i, hh, zoff in ((1, 4, 64), (2, 4, 64), (3, 1, 32)):
        rz, bn, bhn = gate_bias(p[f'gru{i}_bih'], p[f'gru{i}_bhh'], hh, zoff)
        bp.add(f'brz{i}', rz[:, None])
        bp.add(f'bn{i}', bn[:, None])
        bp.add(f'bhn{i}', bhn[:, None])

    bu3 = np.array([p['fc3_b'][d, 0] for d in range(D)], F32)
    bp.add('bu3', bu3[:, None])
    bk = np.zeros(32, F32)
    b6 = np.zeros(64, F32)
    b7 = np.zeros(64, F32)
    for d in range(D):
        for kk in range(2):
            bk[2 * d + kk] = p['fc5b_b'][d, kk]
        for o in range(4):
            b6[16 * o + d] = p['fc6_b'][d, o]
            b7[16 * o + d] = p['fc7_b'][d, o]
    bp.add('bk', bk[:, None])
    bp.add('b6', b6[:, None])
    bp.add('b7', b7[:, None])
    bp.add('beps', np.full((16, 1), 1e-12, F32))

    return wp.pack(), bp.pack(), wp.slots, bp.slots


# ---------------------------------------------------------------------------
# Bass kernel builder
# ---------------------------------------------------------------------------

def _make_bass(sg, wcols, bcols, slots16, slots32):
    """One super-group of sg chunks; bc = sg*CH columns per core."""
    import concourse.bass as bass
    import concourse.tile as tile
    from concourse import bacc, mybir

    f16, f32 = mybir.dt.float16, mybir.dt.float32
    i32 = mybir.dt.int32
    AF = mybir.ActivationFunctionType
    OP = mybir.AluOpType
    bc = sg * CH
    SROWS = 16 * sg

    nc = bacc.Bacc("TRN2", target_bir_lowering=False, debug=False,
                   num_devices=NCORES)

    xin = nc.dram_tensor("xin", [96, bc], f16, kind="ExternalInput")
    wp16 = nc.dram_tensor("wp16", [128, wcols], f16, kind="ExternalInput")
    bp32 = nc.dram_tensor("bp32", [128, bcols], f32, kind="ExternalInput")
    yout = nc.dram_tensor("yout", [176, bc], f16, kind="ExternalOutput")

    with tile.TileContext(nc) as tc, \
         tc.tile_pool(name="const", bufs=1) as constp, \
         tc.tile_pool(name="perm", bufs=1) as perm, \
         tc.tile_pool(name="xinp", bufs=2) as xinp, \
         tc.tile_pool(name="u1p", bufs=2) as u1p, \
         tc.tile_pool(name="sqp", bufs=1) as sqp, \
         tc.tile_pool(name="sbp", bufs=1) as sbp, \
         tc.tile_pool(name="gisp", bufs=2) as gisp, \
         tc.tile_pool(name="smallp", bufs=2) as smallp, \
         tc.tile_pool(name="chainp", bufs=1) as chainp, \
         tc.tile_pool(name="vp", bufs=2) as vp, \
         tc.tile_pool(name="pbig", bufs=2, space="PSUM") as pbig, \
         tc.tile_pool(name="psmall", bufs=2, space="PSUM") as psmall:

        wsb = constp.tile([128, wcols], f16, tag="wsb")
        bsb = constp.tile([128, bcols], f32, tag="bsb")
        nc.sync.dma_start(wsb[:], wp16[:])
        nc.sync.dma_start(bsb[:], bp32[:])

        def W(name):
            k, c, m = slots16[name]
            return wsb[0:k, c:c + m]

        def BI(name):
            k, c, m = slots32[name]
            return bsb[0:k, c:c + 1]

        def ptile(tag, rows, dtype=f16):
            return perm.tile([rows, CH], dtype, tag=tag, name=tag)

        # ss accumulators: chunk ci lives in tile ci//4 at rows 32*(ci%4)
        nsst = (sg + 3) // 4
        sstiles = {}
        for nm in ("ss1", "ss2", "ss3"):
            sstiles[nm] = [perm.tile([32 * min(4, sg - 4 * j), CH], f32,
                                     tag=f"{nm}a{j}", name=f"{nm}a{j}")
                           for j in range(nsst)]
            for t_ in sstiles[nm]:
                nc.gpsimd.memset(t_[:], 1.0)

        def ssrow(nm, ci):
            return sstiles[nm][ci // 4], 32 * (ci % 4)

        U2, U4, QN, G1A, G1B, G2A, G2B, G3, CAT, U3, SNX = \
            {}, {}, {}, {}, {}, {}, {}, {}, {}, {}, {}

        # ---------------- P1 ----------------
        for ci in range(sg):
            col = ci * CH
            ssb = psmall.tile([128, CH], f32, tag="ssbank", bufs=1)
            g1a = pbig.tile([128, CH], f32, tag="gbig")
            g1b = psmall.tile([64, CH], f32, tag="gsmall")
            u2_t = [ptile(f"u2_{ci}_{t}", 128) for t in range(_NU_T)]
            u4_t = [ptile(f"u4_{ci}_{t}", 128) for t in range(_NU_T)]
            U2[ci], U4[ci] = u2_t, u4_t

            for t in range(_NU_T):
                dmin, dmax, klo, khi = _u_tile_span(t)
                K = khi - klo
                xt = xinp.tile([K, CH], f16, tag="xt", name="xt")
                nc.sync.dma_start(xt[:], xin[klo:khi, col:col + CH])
                for which, wname, bname, dest in (
                    (0, f'u1_{t}', f'bu1_{t}', None),
                    (1, f'u2_{t}', f'bu2_{t}', u2_t[t]),
                    (2, f'u4_{t}', f'bu4_{t}', u4_t[t]),
                ):
                    up = pbig.tile([128, CH], f32, tag="upsum", bufs=3,
                                   name="upsum")
                    nc.tensor.matmul(up[:], W(wname), xt[:],
                                     start=True, stop=True)
                    if dest is None:
                        dest = u1p.tile([128, CH], f16, tag="u1t",
                                        name="u1t")
                    if (t + which) % 2 == 0:
                        nc.scalar.activation(dest[:], up[:], AF.Relu,
                                             bias=BI(bname))
                    else:
                        nc.vector.tensor_scalar(dest[:], up[:],
                                                BI(bname), 0.0,
                                                OP.add, OP.max)
                    sq = sqp.tile([128, CH], f16, tag="sqt", name="sqt")
                    eng = nc.vector if which == 0 else nc.gpsimd
                    eng.tensor_tensor(sq[:], dest[:], dest[:], OP.mult)
                    nc.tensor.matmul(
                        ssb[32 * which:32 * which + 16, :],
                        W(f'ss_{t}'), sq[:],
                        start=(t == 0), stop=(t == _NU_T - 1),
                        skip_group_check=True)
                    if which == 0:
                        nc.tensor.matmul(g1a[:], W(f'g1A_{t}'), dest[:],
                                         start=(t == 0),
                                         stop=(t == _NU_T - 1))
                        nc.tensor.matmul(g1b[:], W(f'g1B_{t}'), dest[:],
                                         start=(t == 0),
                                         stop=(t == _NU_T - 1))

            g1a_sb = ptile(f"g1a_{ci}", 128)
            g1b_sb = ptile(f"g1b_{ci}", 64)
            G1A[ci], G1B[ci] = g1a_sb, g1b_sb
            nc.scalar.copy(g1a_sb[:], g1a[:])
            nc.scalar.copy(g1b_sb[:], g1b[:])
            t1a, r1 = ssrow("ss1", ci)
            t2a, r2 = ssrow("ss2", ci)
            t3a, r3 = ssrow("ss3", ci)
            nc.scalar.activation(t1a[r1:r1 + 16, :], ssb[0:16, :],
                                 AF.Identity, bias=BI('beps'))
            nc.vector.tensor_scalar(t2a[r2:r2 + 16, :], ssb[32:48, :],
                                    1e-12, None, OP.add)
            nc.vector.tensor_scalar(t3a[r3:r3 + 16, :], ssb[64:80, :],
                                    1e-12, None, OP.add)

        # ---------------- rsqrt chain ----------------
        def rsqrt_chain(nm, tag):
            # per ss tile: fp32 >= 1e-12 -> fp16 ~ rsqrt(src)
            outs = []
            for j, src in enumerate(sstiles[nm]):
                rows = src.shape[0]
                y0 = chainp.tile([rows, CH], f32, tag="chain", bufs=2,
                                 name="c_y0")
                nc.vector.tensor_scalar(
                    y0.bitcast(i32)[:], src.bitcast(i32)[:], 1, -1,
                    OP.logical_shift_right, OP.bitwise_xor)
                nc.vector.tensor_scalar(
                    y0.bitcast(i32)[:], y0.bitcast(i32)[:],
                    0x5f3759e0, None, OP.add)
                ysq = chainp.tile([rows, CH], f32, tag="chain", bufs=2,
                                  name="c_ysq")
                nc.scalar.activation(ysq[:], y0[:], AF.Square)
                nc.vector.scalar_tensor_tensor(ysq[:], src[:], -0.5, ysq[:],
                                               OP.mult, OP.mult)
                out = perm.tile([rows, CH], f16, tag=f"{tag}{j}",
                                name=f"{tag}{j}")
                nc.vector.scalar_tensor_tensor(out[:], ysq[:], 1.5, y0[:],
                                               OP.add, OP.mult)
                outs.append(out)
            return outs

        s1 = rsqrt_chain("ss1", "s1c")

        def bcast(s, ci, nrows):
            # replicate chunk ci's 16 s values into every 16-row band
            sb = sbp.tile([nrows, CH], f16, tag=f"sb{nrows}",
                          name=f"sb{nrows}")
            src = s[ci // 4]
            r0 = 32 * (ci % 4)
            for h in range(nrows // 16):
                nc.sync.dma_start(sb[16 * h:16 * h + 16, :],
                                  src[r0:r0 + 16, :])
            return sb

        # ---------------- P3: gru1, gi2, ss2-final ----------------
        for ci in range(sg):
            s1b = bcast(s1, ci, 128)
            gisA = gisp.tile([128, CH], f16, tag="gis", bufs=3)
            gisB = gisp.tile([64, CH], f16, tag="gis", bufs=3)
            nc.vector.tensor_tensor(gisA[:], G1A[ci][:], s1b[:], OP.mult)
            nc.vector.tensor_tensor(gisB[:], G1B[ci][:], s1b[0:64, :],
                                    OP.mult)
            rz = smallp.tile([128, CH], f16, tag="rz")
            nc.scalar.activation(rz[:], gisA[:], AF.Sigmoid, bias=BI('brz1'))
            cpre = smallp.tile([64, CH], f16, tag="cpre", bufs=1)
            nc.vector.scalar_tensor_tensor(cpre[:], rz[0:64, :], BI('bhn1'),
                                           gisB[:], OP.mult, OP.add)
            cand = smallp.tile([64, CH], f16, tag="cand", bufs=1)
            nc.scalar.activation(cand[:], cpre[:], AF.Tanh, bias=BI('bn1'))
            zm = smallp.tile([64, CH], f16, tag="zm", bufs=1)
            nc.vector.tensor_scalar(zm[:], rz[64:128, :], -1.0, 1.0,
                                    OP.mult, OP.add)
            qn = ptile(f"qn_{ci}", 64)
            QN[ci] = qn
            nc.vector.tensor_tensor(qn[:], zm[:], cand[:], OP.mult)

            qsq = smallp.tile([64, CH], f16, tag="qsq", bufs=1)
            nc.gpsimd.tensor_tensor(qsq[:], qn[:], qn[:], OP.mult)
            t2a, r2 = ssrow("ss2", ci)
            q2p = psmall.tile([128, CH], f32, tag="gsmall")
            nc.tensor.matmul(q2p[r2:r2 + 16, :], W('ss_q'), qsq[:],
                             start=True, stop=True,
                             tile_position=(0, r2) if r2 == 96 else None)
            nc.vector.tensor_tensor(t2a[r2:r2 + 16, :],
                                    t2a[r2:r2 + 16, :], q2p[r2:r2 + 16, :],
                                    OP.add)

            g2a = pbig.tile([128, CH], f32, tag="gbig")
            g2b = psmall.tile([64, CH], f32, tag="gsmall")
            for t in range(_NU_T):
                nc.tensor.matmul(g2a[:], W(f'g2A_{t}'), U2[ci][t][:],
                                 start=(t == 0), stop=False)
            nc.tensor.matmul(g2a[:], W('g2A_q'), qn[:],
                             start=False, stop=True)
            for t in range(_NU_T):
                nc.tensor.matmul(g2b[:], W(f'g2B_{t}'), U2[ci][t][:],
                                 start=(t == 0), stop=False)
            nc.tensor.matmul(g2b[:], W('g2B_q'), qn[:],
                             start=False, stop=True)
            g2a_sb = ptile(f"g2a_{ci}", 128)
            g2b_sb = ptile(f"g2b_{ci}", 64)
            G2A[ci], G2B[ci] = g2a_sb, g2b_sb
            nc.scalar.copy(g2a_sb[:], g2a[:])
            nc.vector.tensor_copy(g2b_sb[:], g2b[:])

        # ---------------- P4/P5: gru2, u3, gi3, ss3-final -------------
        s2 = rsqrt_chain("ss2", "s2c")

        for ci in range(sg):
            s2b = bcast(s2, ci, 128)
            gisA = gisp.tile([128, CH], f16, tag="gis", bufs=3)
            gisB = gisp.tile([64, CH], f16, tag="gis", bufs=3)
            nc.vector.tensor_tensor(gisA[:], G2A[ci][:], s2b[:], OP.mult)
            nc.vector.tensor_tensor(gisB[:], G2B[ci][:], s2b[0:64, :],
                                    OP.mult)
            rz = smallp.tile([128, CH], f16, tag="rz")
            nc.scalar.activation(rz[:], gisA[:], AF.Sigmoid, bias=BI('brz2'))
            cpre = smallp.tile([64, CH], f16, tag="cpre", bufs=1)
            nc.vector.scalar_tensor_tensor(cpre[:], rz[0:64, :], BI('bhn2'),
                                           gisB[:], OP.mult, OP.add)
            cand = smallp.tile([64, CH], f16, tag="cand", bufs=1)
            nc.scalar.activation(cand[:], cpre[:], AF.Tanh, bias=BI('bn2'))
            zm = smallp.tile([64, CH], f16, tag="zm", bufs=1)
            nc.vector.tensor_scalar(zm[:], rz[64:128, :], -1.0, 1.0,
                                    OP.mult, OP.add)
            cat = ptile(f"cat_{ci}", 80)
            CAT[ci] = cat
            nc.vector.tensor_tensor(cat[0:64, :], zm[:], cand[:], OP.mult)

            u3p = psmall.tile([16, CH], f32, tag="gsmall")
            nc.tensor.matmul(u3p[:], W('fc3'), cat[0:64, :],
                             start=True, stop=True)
            u3 = ptile(f"u3_{ci}", 16)
            U3[ci] = u3
            nc.scalar.activation(u3[:], u3p[:], AF.Relu, bias=BI('bu3'))
            u3sq = smallp.tile([16, CH], f16, tag="qsq", bufs=1)
            nc.vector.tensor_tensor(u3sq[:], u3[:], u3[:], OP.mult)
            t3a, r3 = ssrow("ss3", ci)
            q3p = psmall.tile([128, CH], f32, tag="gsmall")
            nc.tensor.matmul(q3p[r3:r3 + 16, :], W('ss_u3'), u3sq[:],
                             start=True, stop=True,
                             tile_position=(0, r3) if r3 == 96 else None)
            nc.vector.tensor_tensor(t3a[r3:r3 + 16, :],
                                    t3a[r3:r3 + 16, :], q3p[r3:r3 + 16, :],
                                    OP.add)

            # gi3 layout: r @ 0:16, z @ 32:48, n @ 64:80
            g3p = psmall.tile([80, CH], f32, tag="gsmall")
            for t in range(_NU_T):
                nc.tensor.matmul(g3p[:], W(f'g3_{t}'), U4[ci][t][:],
                                 start=(t == 0), stop=False)
            nc.tensor.matmul(g3p[:], W('g3_u3'), u3[:],
                             start=False, stop=True)
            g3_sb = ptile(f"g3_{ci}", 80)
            G3[ci] = g3_sb
            nc.vector.tensor_copy(g3_sb[:], g3p[:])

        # ---------------- P6/P7: gru3, fc5, fc6, fc7, out -------------
        s3 = rsqrt_chain("ss3", "s3c")

        for ci in range(sg):
            col = ci * CH
            cat = CAT[ci]
            s3b = bcast(s3, ci, 80)

            gis = gisp.tile([48, CH], f16, tag="gis", bufs=3)
            gisn = gisp.tile([16, CH], f16, tag="gis", bufs=3)
            nc.vector.tensor_tensor(gis[:], G3[ci][0:48, :], s3b[0:48, :],
                                    OP.mult)
            nc.vector.tensor_tensor(gisn[:], G3[ci][64:80, :],
                                    s3b[64:80, :], OP.mult)
            rz = smallp.tile([48, CH], f16, tag="rz")
            nc.scalar.activation(rz[:], gis[:], AF.Sigmoid,
                                 bias=BI('brz3'))
            cpre = smallp.tile([16, CH], f16, tag="cpre", bufs=1)
            nc.vector.scalar_tensor_tensor(cpre[:], rz[0:16, :], BI('bhn3'),
                                           gisn[:], OP.mult, OP.add)
            cand = smallp.tile([16, CH], f16, tag="cand", bufs=1)
            nc.scalar.activation(cand[:], cpre[:], AF.Tanh, bias=BI('bn3'))
            zm = smallp.tile([16, CH], f16, tag="zm", bufs=1)
            nc.vector.tensor_scalar(zm[:], rz[32:48, :], -1.0, 1.0,
                                    OP.mult, OP.add)
            nc.vector.tensor_tensor(cat[64:80, :], zm[:], cand[:], OP.mult)

            snk = smallp.tile([64, CH], f16, tag="snk", bufs=1)
            nc.sync.dma_start(snk[0:16, :], cat[64:80, :])
            nc.sync.dma_start(snk[16:32, :], cat[64:80, :])

            kp = psmall.tile([32, CH], f32, tag="gsmall")
            for t in range(_NV_T):
                vps = pbig.tile([128, CH], f32, tag="gbig")
                nc.tensor.matmul(vps[:], W(f'v_{t}'), cat[0:80, :],
                                 start=True, stop=True)
                vsb = vp.tile([128, CH], f16, tag="vsb", name="vsb")
                if t % 2 == 0:
                    nc.scalar.activation(vsb[:], vps[:], AF.Relu,
                                         bias=BI(f'bv_{t}'))
                else:
                    nc.vector.tensor_scalar(vsb[:], vps[:], BI(f'bv_{t}'),
                                            0.0, OP.add, OP.max)
                nc.tensor.matmul(kp[:], W(f'k5_{t}'), vsb[:],
                                 start=(t == 0), stop=(t == _NV_T - 1))
            nc.scalar.activation(snk[32:64, :], kp[:], AF.Identity,
                                 bias=BI('bk'))

            u6p = psmall.tile([64, CH], f32, tag="gsmall")
            nc.tensor.matmul(u6p[:], W('u6w'), snk[:],
                             start=True, stop=True)
            u6 = smallp.tile([64, CH], f16, tag="u6", bufs=1)
            nc.scalar.activation(u6[:], u6p[:], AF.Relu, bias=BI('b6'))
            u7p = psmall.tile([64, CH], f32, tag="gsmall")
            nc.tensor.matmul(u7p[:], W('u7a'), cat[0:64, :],
                             start=True, stop=False)
            nc.tensor.matmul(u7p[:], W('u7b'), u6[:],
                             start=False, stop=True)
            snx = ptile(f"snx_{ci}", 64)
            SNX[ci] = snx
            nc.scalar.activation(snx[:], u7p[:], AF.Relu, bias=BI('b7'))

            nc.sync.dma_start(yout[0:32, col:col + CH], snk[32:64, :])
            nc.sync.dma_start(yout[32:96, col:col + CH], QN[ci][:])
            nc.sync.dma_start(yout[96:160, col:col + CH], snx[:])
            nc.sync.dma_start(yout[160:176, col:col + CH], cat[64:80, :])

    nc.compile()
    return nc


# ---------------------------------------------------------------------------
# Host orchestration
# ---------------------------------------------------------------------------

_cache = {}
LAST_RESULT = None


def _get_bass(sg, inputs):
    key = ('nc', sg) + tuple(id(inputs[k]) for k in _PARAM_KEYS)
    if key in _cache:
        return _cache[key]
    p = {k: np.asarray(inputs[k], dtype=F32) for k in _PARAM_KEYS}
    w16, b32, s16, s32 = _build_packs(p)
    nc = _make_bass(sg, w16.shape[1], b32.shape[1], s16, s32)
    _cache.clear()
    _cache[key] = (nc, w16, b32)
    return _cache[key]


def _pack_xin(inputs):
    key = ('xin',) + tuple(id(inputs[k]) for k in
                           ('del_y_til', 'del_y', 'del_x_til', 'del_x_hat'))
    if key in _cache:
        return _cache[key]
    x = np.concatenate([
        np.asarray(inputs['del_y_til'], F32),
        np.asarray(inputs['del_y'], F32),
        np.asarray(inputs['del_x_til'], F32),
        np.asarray(inputs['del_x_hat'], F32),
    ], axis=2)  # [B, D, 6], f order: yt, y, xt0, xt1, xh0, xh1
    xin = np.ascontiguousarray(x.reshape(B, 96).T.astype(F16))  # [96, B]
    _cache[key] = xin
    return xin


def _unpack_out(youts):
    parts = []
    for y in youts:
        yt = y.astype(F32).T  # [bc, 176]
        bcn = yt.shape[0]
        K = yt[:, 0:32].reshape(bcn, D, 2)
        Qn = yt[:, 32:96].reshape(bcn, 4, D).transpose(0, 2, 1)
        Sx = yt[:, 96:160].reshape(bcn, 4, D).transpose(0, 2, 1)
        Sn = yt[:, 160:176].reshape(bcn, D, 1)
        parts.append(np.concatenate([K, Qn, Sx, Sn], axis=2))
    return np.ascontiguousarray(np.concatenate(parts, axis=0))


def kernel(**inputs):
    zero_state = not (np.any(inputs['Q']) or np.any(inputs['Sigma'])
                      or np.any(inputs['S']))
    if not zero_state:
        return _kernel_jax_fallback(**inputs)

    from concourse.bass_utils import run_bass_kernel_spmd

    sg = 8
    percore = B // NCORES
    assert percore == sg * CH

    nc, w16, b32 = _get_bass(sg, inputs)
    xin = _pack_xin(inputs)

    in_maps = []
    for c in range(NCORES):
        in_maps.append({
            'xin': np.ascontiguousarray(
                xin[:, c * percore:(c + 1) * percore]),
            'wp16': w16,
            'bp32': b32,
        })
    res = run_bass_kernel_spmd(nc, in_maps, list(range(NCORES)))
    global LAST_RESULT
    LAST_RESULT = res
    youts = [res.results[c]['yout'] for c in range(NCORES)]
    return _unpack_out(youts)


# ---------------------------------------------------------------------------
# General-case fallback (recurrent states nonzero): jax pmap, correct but slow
# ---------------------------------------------------------------------------

def _kernel_jax_fallback(**inputs):
    import jax
    import jax.numpy as jnp

    def _lin(x, w, b):
        return jnp.einsum('bdi,doi->bdo', x, w) + b

    def _fc(x, w, b):
        return jax.nn.relu(_lin(x, w, b))

    def _l2norm(x):
        nrm = jnp.sqrt(jnp.sum(x * x, axis=-1, keepdims=True))
        return x / jnp.maximum(nrm, EPS)

    def _gru_step(x, h, wih, whh, bih, bhh):
        gi = jnp.einsum('bdi,dgi->bdg', x, wih) + bih
        gh = jnp.einsum('bdh,dgh->bdg', h, whh) + bhh
        ir, iz, i_n = jnp.split(gi, 3, axis=-1)
        hr, hz, h_n = jnp.split(gh, 3, axis=-1)
        r = jax.nn.sigmoid(ir + hr)
        z = jax.nn.sigmoid(iz + hz)
        cand = jnp.tanh(i_n + r * h_n)
        return (1.0 - z) * cand + z * h

    def _forward(batch, params):
        (del_y_til, del_y, del_x_til, del_x_hat, Q, Sigma, S) = batch
        p = dict(zip(_PARAM_KEYS, params))
        in1 = _l2norm(_fc(del_x_hat, p['fc1_w'], p['fc1_b']))
        Qn = _gru_step(in1, Q, p['gru1_wih'], p['gru1_whh'],
                       p['gru1_bih'], p['gru1_bhh'])
        in2 = _l2norm(jnp.concatenate(
            [Qn, _fc(del_x_til, p['fc2_w'], p['fc2_b'])], axis=-1))
        Sigman = _gru_step(in2, Sigma, p['gru2_wih'], p['gru2_whh'],
                           p['gru2_bih'], p['gru2_bhh'])
        in3 = _l2norm(jnp.concatenate([
            _fc(Sigman, p['fc3_w'], p['fc3_b']),
            _fc(jnp.concatenate([del_y_til, del_y], axis=-1),
                p['fc4_w'], p['fc4_b'])], axis=-1))
        Sn = _gru_step(in3, S, p['gru3_wih'], p['gru3_whh'],
                       p['gru3_bih'], p['gru3_bhh'])
        cat_ss = jnp.concatenate([Sigman, Sn], axis=-1)
        K = _lin(jax.nn.relu(_lin(cat_ss, p['fc5a_w'], p['fc5a_b'])),
                 p['fc5b_w'], p['fc5b_b'])
        Sigma_next = _fc(jnp.concatenate(
            [Sigman, _fc(jnp.concatenate([Sn, K], axis=-1),
                         p['fc6_w'], p['fc6_b'])],
            axis=-1), p['fc7_w'], p['fc7_b'])
        return jnp.concatenate([K, Qn, Sigma_next, Sn], axis=-1)

    devs = jax.devices()[:NCORES]
    pm = jax.pmap(_forward, devices=devs)
    batch_keys = ['del_y_til', 'del_y', 'del_x_til', 'del_x_hat',
                  'Q', 'Sigma', 'S']
    batch = [np.asarray(inputs[k]).reshape(NCORES, B // NCORES,
                                           *inputs[k].shape[1:])
             for k in batch_keys]
    params = [np.broadcast_to(np.asarray(inputs[k]),
                              (NCORES,) + inputs[k].shape)
              for k in _PARAM_KEYS]
    out = pm(batch, params)
    return np.asarray(out).reshape(B, D, 11)
